# revision 7
# baseline (speedup 1.0000x reference)
"""Trainium2 Bass kernel for nn_AdditiveAttention (B=8, Q=512, K=1024, D=128, H=64).

Strategy: data-parallel over batch (1 batch element per NeuronCore, 8 cores).

Per-core math (q in [0,512), k in [0,1024), h in [0,64)):
    qh = queries @ W_q            [Q, H]
    kh = keys @ W_k               [K, H]
    scores[q, k] = sum_h w_v[h] * tanh(qh[q,h] + kh[k,h])
    attn = softmax_k(mask(scores));  out = attn @ values

Device-side layout: everything is computed in the transposed [k, q]
orientation so the exp output feeds the attention*values matmul directly
(k on partitions = contraction dim) with zero transposes of the big
intermediates. Two k's are packed per 128-partition tile (H=64), so the
tanh feature tile for "k-pair" i is
    feat[64*j + h, q] = tanh(qh[q,h] + kh[2i+j, h]),  j in {0,1}
built by a DVE per-partition-scalar add (qh2 + khp[:, i]) and one big
ACT Tanh. A block-diagonal stationary matrix per pair reduces over h on
the PE (float32r = full-rate fp32) accumulating transposed scores
[128 k, 512 q] per k-tile in PSUM. Masking rides for free as the
per-partition bias of the Exp activation (bias 0 or -1e6; exp -> exact 0).
Softmax normalization is deferred: sums over k via a ones-vector matmul,
reciprocal on the tiny [Q] vector, applied after the final transpose.
"""

import numpy as np

B, Q, K = 8, 512, 1024
DQ, DK, DV, H = 128, 128, 128, 64
MASK_VAL = -1000000.0

N_CORES = 8
KT = K // 128          # 8 k-tiles of 128 keys
PAIRS = K // 2         # 512 k-pairs
PPC = 8                # pairs per tanh chunk
CHUNK_FD = PPC * Q     # 4096
PAIRS_PER_KT = 64      # pairs per k-tile
CHUNKS_PER_KT = PAIRS_PER_KT // PPC  # 8
QT = Q // 128          # 4 q-tiles

_CACHE = {}


def _build_nc():
    import concourse.bacc as bacc
    import concourse.tile as tile
    from concourse import mybir

    f32 = mybir.dt.float32
    f32r = mybir.dt.float32r

    nc = bacc.Bacc("TRN2", target_bir_lowering=False, debug=False,
                   num_devices=N_CORES)

    qh2_d = nc.dram_tensor("qh2", [128, Q], f32, kind="ExternalInput")
    khp_d = nc.dram_tensor("khp", [128, PAIRS], f32, kind="ExternalInput")
    vals_d = nc.dram_tensor("vals", [K, DV], f32, kind="ExternalInput")
    mask_d = nc.dram_tensor("maskT", [128, KT], f32, kind="ExternalInput")
    wvb_d = nc.dram_tensor("wvb", [128, PAIRS_PER_KT * 128], f32r,
                           kind="ExternalInput")
    ident_d = nc.dram_tensor("ident", [128, 128], f32, kind="ExternalInput")
    out_d = nc.dram_tensor("out", [Q, DV], f32, kind="ExternalOutput")

    Tanh = mybir.ActivationFunctionType.Tanh
    Exp = mybir.ActivationFunctionType.Exp

    with tile.TileContext(nc) as tc:
        with (
            tc.tile_pool(name="const", bufs=1) as cpool,
            tc.tile_pool(name="attn", bufs=1) as apool,
            tc.tile_pool(name="fin", bufs=2) as fin_pool,
            tc.tile_pool(name="fout", bufs=2) as fout_pool,
            tc.tile_pool(name="small", bufs=1) as spool,
            tc.tile_pool(name="osb", bufs=2) as opool,
            tc.tile_pool(name="ps_scores", bufs=2, space="PSUM") as ps_s,
            tc.tile_pool(name="ps_sums", bufs=1, space="PSUM") as ps_sum,
            tc.tile_pool(name="ps_outT", bufs=1, space="PSUM") as ps_o,
            tc.tile_pool(name="ps_rt", bufs=1, space="PSUM") as ps_rt,
            tc.tile_pool(name="ps_oq", bufs=2, space="PSUM") as ps_oq,
        ):
            # ---- load constants/inputs ----
            qh2 = cpool.tile([128, Q], f32)
            nc.sync.dma_start(qh2[:], qh2_d[:])
            khp = cpool.tile([128, PAIRS], f32)
            nc.sync.dma_start(khp[:], khp_d[:])
            maskT = cpool.tile([128, KT], f32)
            nc.sync.dma_start(maskT[:], mask_d[:])
            ident = cpool.tile([128, 128], f32)
            nc.sync.dma_start(ident[:], ident_d[:])
            wvb = cpool.tile([128, PAIRS_PER_KT * 128], f32r)
            nc.sync.dma_start(wvb[:], wvb_d[:])
            vals = cpool.tile([128, KT * 128], f32)
            for t in range(KT):
                nc.sync.dma_start(vals[:, t * 128:(t + 1) * 128],
                                  vals_d[t * 128:(t + 1) * 128, :])
            ones_col = cpool.tile([128, 1], f32)
            nc.vector.memset(ones_col[:], 1.0)

            attn = apool.tile([128, KT * Q], f32)

            # ---- main loop: tanh features + score reduction ----
            for t in range(KT):
                ps = ps_s.tile([128, Q], f32)
                for c in range(CHUNKS_PER_KT):
                    fin = fin_pool.tile([128, CHUNK_FD], f32)
                    for j in range(PPC):
                        pair = t * PAIRS_PER_KT + c * PPC + j
                        nc.vector.tensor_scalar_add(
                            fin[:, j * Q:(j + 1) * Q], qh2[:],
                            khp[:, pair:pair + 1])
                    fout = fout_pool.tile([128, CHUNK_FD], f32r)
                    nc.scalar.activation(fout[:], fin[:], Tanh)
                    for j in range(PPC):
                        ii = c * PPC + j
                        nc.tensor.matmul(
                            ps[:],
                            wvb[:, ii * 128:(ii + 1) * 128],
                            fout[:, j * Q:(j + 1) * Q],
                            start=(ii == 0), stop=(ii == PAIRS_PER_KT - 1))
                # exp with additive mask as per-partition bias
                nc.scalar.activation(attn[:, t * Q:(t + 1) * Q], ps[:], Exp,
                                     bias=maskT[:, t:t + 1])

            # ---- softmax denominator + attn @ values ----
            ps_sums = ps_sum.tile([1, Q], f32)
            for t in range(KT):
                nc.tensor.matmul(ps_sums[:], ones_col[:],
                                 attn[:, t * Q:(t + 1) * Q],
                                 start=(t == 0), stop=(t == KT - 1))
            ps_out = ps_o.tile([128, Q], f32)
            for t in range(KT):
                nc.tensor.matmul(ps_out[:],
                                 vals[:, t * 128:(t + 1) * 128],
                                 attn[:, t * Q:(t + 1) * Q],
                                 start=(t == 0), stop=(t == KT - 1))

            # ---- normalize + transpose back to [q, v] ----
            sums_sb = spool.tile([1, Q], f32)
            nc.vector.tensor_copy(sums_sb[:], ps_sums[:])
            rt = ps_rt.tile([128, QT], f32)
            for qt in range(QT):
                nc.tensor.transpose(rt[:, qt:qt + 1],
                                    sums_sb[0:1, qt * 128:(qt + 1) * 128],
                                    ident[0:1, 0:1],
                                    )
            recip = spool.tile([128, QT], f32)
            nc.vector.reciprocal(recip[:], rt[:])

            outT = spool.tile([128, Q], f32)
            nc.vector.tensor_copy(outT[:], ps_out[:])
            for qt in range(QT):
                oq = ps_oq.tile([128, 128], f32)
                nc.tensor.transpose(oq[:], outT[:, qt * 128:(qt + 1) * 128],
                                    ident[:])
                osb = opool.tile([128, 128], f32)
                nc.vector.tensor_scalar_mul(osb[:], oq[:],
                                            recip[:, qt:qt + 1])
                nc.sync.dma_start(out_d[qt * 128:(qt + 1) * 128, :], osb[:])

    nc.compile()
    return nc


def _get_nc():
    if "nc" not in _CACHE:
        _CACHE["nc"] = _build_nc()
    return _CACHE["nc"]


def _round_tf32(x):
    """Round fp32 mantissa to 10 bits (nearest-even) — fp32r/TF32 format."""
    xi = np.asarray(x, dtype=np.float32).view(np.uint32)
    add = ((xi >> np.uint32(13)) & np.uint32(1)) + np.uint32(0x0FFF)
    xi = (xi + add) & np.uint32(0xFFFFE000)
    return xi.view(np.float32)


def _host_prep(queries, keys, values, valid_lens, W_q, W_k, w_v):
    """Build the per-core input maps (shard over batch)."""
    queries = np.asarray(queries, dtype=np.float32)
    keys = np.asarray(keys, dtype=np.float32)
    values = np.asarray(values, dtype=np.float32)
    valid_lens = np.asarray(valid_lens)
    W_q = np.asarray(W_q, dtype=np.float32)
    W_k = np.asarray(W_k, dtype=np.float32)
    w_v = np.asarray(w_v, dtype=np.float32)

    # shared across cores
    wvb = np.zeros((128, PAIRS_PER_KT * 128), dtype=np.float32)
    w_v_r = _round_tf32(w_v)
    for ii in range(PAIRS_PER_KT):
        wvb[0:H, ii * 128 + 2 * ii] = w_v_r
        wvb[H:128, ii * 128 + 2 * ii + 1] = w_v_r
    ident = np.eye(128, dtype=np.float32)
    karr = np.arange(K, dtype=np.int64).reshape(KT, 128).T  # [128, KT]

    in_maps = []
    for b in range(B):
        qh = queries[b] @ W_q                      # [Q, H]
        kh = keys[b] @ W_k                         # [K, H]
        qh2 = np.concatenate([qh.T, qh.T], axis=0)  # [128, Q]
        khT3 = kh.T.reshape(H, PAIRS, 2)
        khp = np.concatenate([khT3[:, :, 0], khT3[:, :, 1]], axis=0)  # [128, PAIRS]
        vl = int(valid_lens[b])
        maskT = np.where(karr < vl, 0.0, MASK_VAL).astype(np.float32)
        in_maps.append({
            "qh2": np.ascontiguousarray(qh2),
            "khp": np.ascontiguousarray(khp),
            "vals": np.ascontiguousarray(values[b]),
            "maskT": np.ascontiguousarray(maskT),
            "wvb": wvb,
            "ident": ident,
        })
    return in_maps


def kernel(queries, keys, values, valid_lens, W_q, W_k, w_v):
    from concourse.bass_utils import run_bass_kernel_spmd

    nc = _get_nc()
    in_maps = _host_prep(queries, keys, values, valid_lens, W_q, W_k, w_v)
    res = run_bass_kernel_spmd(nc, in_maps, list(range(N_CORES)))
    out = np.stack([res.results[i]["out"] for i in range(N_CORES)], axis=0)
    return out.astype(np.float32)


if __name__ == "__main__":
    rng = np.random.default_rng(0)
    inputs = {
        "queries": rng.standard_normal((B, Q, DQ), dtype=np.float32),
        "keys": rng.standard_normal((B, K, DK), dtype=np.float32),
        "values": rng.standard_normal((B, K, DV), dtype=np.float32),
        "valid_lens": rng.integers(1, K + 1, size=(B,), dtype=np.int32),
        "W_q": (rng.standard_normal((DQ, H)) / np.sqrt(DQ)).astype(np.float32),
        "W_k": (rng.standard_normal((DK, H)) / np.sqrt(DK)).astype(np.float32),
        "w_v": (rng.standard_normal((H,)) / np.sqrt(H)).astype(np.float32),
    }
    out = kernel(**inputs)
    print("out", out.shape, out.dtype)
